# revision 21
# baseline (speedup 1.0000x reference)
"""Trainium2 Bass kernel for nn_Attention_80367428043370.

Math (the reference reduces to this):
  q  = W @ x  (1x1 conv per spatial position)
  kv = conv1x1(pad(x)) = pad(q)
  logits[c,h,w,di,dj] = q[c,h,w] * (kvp[c,h+di,w+dj] + rel[c,di,dj])
  attn = softmax over dj (size 3); out = sum_di attn[...,di,di]*kvp[c,h+di,w+di]

Per-row softmax-ratio form (6 exps instead of 9):
  sigma_di = 1/(1 + sum_{dj!=di} exp(q*(kvp[.,h+di,w+dj]-kvp[.,h+di,w+di]+dr)))
  out      = sum_di sigma_di * kvp[.,h+di,w+di],  dr = rel[c,di,dj]-rel[c,di,di]

Column differences of kvp are shift-shared planes G1/G2.  All elementwise
work is bf16 (DVE 2x modes) except the reciprocal (internal f32).  A custom
DVE op fuses s=1+e0+e1 with an approximate reciprocal (seed + 1 NR step).
Even/odd column-phase copies of the padded plane keep every strided read
4B-aligned so bf16 2x perf modes engage.

Sharding: data-parallel over batch B=8 -> one batch item per NeuronCore.
"""

import sys

for p in ("/opt/trn_rl_repo", "/opt/pypackages"):
    if p not in sys.path:
        sys.path.insert(0, p)

import numpy as np

import concourse.bass as bass

import concourse.bacc as bacc
import concourse.mybir as mybir
import concourse.tile as tile
import concourse.dve_ops as dve_ops_mod
from concourse.dve_ops import DveOp
from concourse.dve_spec import Spec, Src0, Src1, C0, C1, C2, Bin, AluOp
from concourse.bass_utils import run_bass_kernel_spmd

F32 = mybir.dt.float32
BF16 = mybir.dt.bfloat16
AF = mybir.ActivationFunctionType
OP = mybir.AluOpType

B, C, H, W = 8, 256, 64, 64
HW = H * W
NCORES = 8

RECIP_C0 = -0.23549792
RECIP_C1 = 2.0017324


def _register_sumrecip():
    """sigma = approx 1/(imm2 + in0 + in1): BITWISE_NOT seed + 1 NR step."""
    name = "SUM1_RECIP_NR1"
    if name in dve_ops_mod._SUB_OPCODE_FOR_NAME:
        return next(o for o in dve_ops_mod.OPS if o.name == name)
    _s = (Src0 + Src1) + C2
    _not = Bin(AluOp.BITWISE_NOT, _s, _s)
    _y0 = _not * C0
    body = _y0 * (C1 - _s * _y0)

    def _ref(in0, in1, s0, s1, imm2):
        s = in0.astype(np.float32) + in1.astype(np.float32) + np.float32(imm2)
        nx = (~s.view(np.int32)).view(np.float32)
        y0 = nx * np.float32(s0)
        return (y0 * (np.float32(s1) - s * y0)).astype(np.float32)

    op = DveOp(
        name,
        Spec(body=body, reference=_ref),
        subdim=False,
        uops_sha={"v3": "994dd8b3ea1c7e4c", "v4": "a9846cbf3526a936"},
    )
    dve_ops_mod.OPS.append(op)
    dve_ops_mod._SUB_OPCODE_FOR_NAME[op.name] = (
        max(dve_ops_mod._SUB_OPCODE_FOR_NAME.values()) + 1
    )
    dve_ops_mod.CUSTOM_DVE_SPECS[op.name] = op.spec
    return op


SUMRECIP = _register_sumrecip()


def _register_simple(name, body, ref):
    """Register a custom DVE op, self-pinning its uop sha (deterministic
    within a process; the pin is a drift guard, not an external contract)."""
    if name in dve_ops_mod._SUB_OPCODE_FOR_NAME:
        return next(o for o in dve_ops_mod.OPS if o.name == name)
    from concourse.dve_spec import lower
    from concourse.dve_ops import has_src1
    from concourse.dve_uop import DveOpSpec
    spec = Spec(body=body, reference=ref)
    shas = {}
    for ver in ("v3", "v4"):
        uops = lower(spec, ver=ver)
        shas[ver] = DveOpSpec(name=name, uops=uops,
                              rd1_en=has_src1(spec)).sha(ver)
    op = DveOp(name, spec, subdim=False, uops_sha=shas)
    dve_ops_mod.OPS.append(op)
    dve_ops_mod._SUB_OPCODE_FOR_NAME[op.name] = (
        max(dve_ops_mod._SUB_OPCODE_FOR_NAME.values()) + 1
    )
    dve_ops_mod.CUSTOM_DVE_SPECS[op.name] = op.spec
    return op


# (G + dr) * q  /  (dr - G) * q ; dr rides the per-partition scalar slot s0.
FARG_P = _register_simple(
    "FARG_P_ANT", (Src0 + C0) * Src1,
    lambda in0, in1, s0, s1, imm2: ((in0.astype(np.float32) + s0)
                                    * in1.astype(np.float32)))
FARG_N = _register_simple(
    "FARG_N_ANT", (C0 - Src0) * Src1,
    lambda in0, in1, s0, s1, imm2: ((s0 - in0.astype(np.float32))
                                    * in1.astype(np.float32)))
CSUB = _register_simple(
    "CSUB_ANT", Src0 - Src1,
    lambda in0, in1, s0, s1, imm2: in0.astype(np.float32) - in1.astype(np.float32))
CMUL = _register_simple(
    "CMUL_ANT", Src0 * Src1,
    lambda in0, in1, s0, s1, imm2: in0.astype(np.float32) * in1.astype(np.float32))
CADD = _register_simple(
    "CADD_ANT", Src0 + Src1,
    lambda in0, in1, s0, s1, imm2: in0.astype(np.float32) + in1.astype(np.float32))
CCOPY = _register_simple(
    "CCOPY_ANT", Src0 + C0,
    lambda in0, in1, s0, s1, imm2: in0.astype(np.float32) + s0)

# pair table: (pair_idx, di, plane, row_off, col_off_in_phase, positive_sign)
# planes are phase-separated so every read starts 4B-aligned (bf16).
# arg(di,dj) = q * (sgn*G[...] + dr_pair);  neg pairs compute (dr - G)*q.
PAIRS = [
    (0, 0, "g1e", 0, 0, True),   # (0,1):  +G1[h+0, w+0]
    (1, 0, "g2e", 0, 0, True),   # (0,2):  +G2[h+0, w+0]
    (2, 1, "g1e", 1, 0, False),  # (1,0):  -G1[h+1, w+0]
    (3, 1, "g1o", 1, 0, True),   # (1,2):  +G1[h+1, w+1]
    (4, 2, "g2e", 2, 0, False),  # (2,0):  -G2[h+2, w+0]
    (5, 2, "g1o", 2, 0, False),  # (2,1):  -G1[h+2, w+1]
]

CH = 32  # pointwise chunk height (rows)
BANDS = (((0, 16), (16, 24), (40, 24)), ((0, 24), (24, 24), (48, 16)))
A_AHEAD = 2        # how many A-stages run ahead of their B-stage
LAST_FAST = True   # final band's combine on DVE (short drain tail)
XC = 4             # x DMA column chunks per ib
OT1_COPY_ENG = "dve"  # kvpe copies for the second channel half
ACC_ENG = "pool"   # acc/accf adds (pool add ~1.05 ns/elem on HW)
KVPO_MODE = "dma"  # odd-phase production: "dma" | "act" | "pool"
M1_ENG = "pool"    # m1 = sig1*v1
E2_REUSE = True
TS_ACT = (0, 3)    # pair indices whose t2 build runs on ACT (Identity affine)
WORK_BUFS = 2
MM_BUFS = 1
G_BUFS = 1
SIG_BUFS = 1


def _build(reps=1):
    nc = bacc.Bacc("TRN2", target_bir_lowering=False, debug=False)

    x_ext = nc.dram_tensor("x", [C, HW], BF16, kind="ExternalInput")
    wt_ext = nc.dram_tensor("wt", [C, C], BF16, kind="ExternalInput")  # W.T [cin,cout]
    dr_ext = nc.dram_tensor("dr", [C, 6], F32, kind="ExternalInput")
    out_ext = nc.dram_tensor("out", [C, HW], BF16, kind="ExternalOutput")

    with tile.TileContext(nc) as tc:
        with (
            tc.tile_pool(name="const", bufs=1) as const,
            tc.tile_pool(name="planes", bufs=2) as planes,
            tc.tile_pool(name="psum", bufs=8, space="PSUM") as psum,
            tc.tile_pool(name="work", bufs=WORK_BUFS) as work,
            tc.tile_pool(name="gp", bufs=G_BUFS) as gpool,
            tc.tile_pool(name="mm", bufs=MM_BUFS) as mmp,
            tc.tile_pool(name="sigp", bufs=max(SIG_BUFS, 1)) as sigpool,
        ):
            # x in 4 column-chunk tiles per ib so matmuls can start after the
            # first 256KB lands; all x loads serialized on one queue so early
            # chunks get full DMA bandwidth and finish first.
            x_sb = [[None] * XC for _ in range(2)]
            for xc in range(XC):
                for ib in range(2):
                    t = const.tile([128, HW // XC], BF16, tag=f"x{ib}_{xc}")
                    sl = slice(xc * (HW // XC), (xc + 1) * (HW // XC))
                    nc.sync.dma_start(
                        out=t, in_=x_ext.ap()[ib * 128:(ib + 1) * 128, sl])
                    x_sb[ib][xc] = t
            wt_sb = []
            for ib in range(2):
                t = const.tile([128, C], BF16, tag=f"wt{ib}")
                nc.scalar.dma_start(out=t, in_=wt_ext.ap()[ib * 128:(ib + 1) * 128, :])
                wt_sb.append(t)
            dr_sb = []
            for ot in range(2):
                t = const.tile([128, 6], F32, tag=f"dr{ot}")
                nc.gpsimd.dma_start(out=t, in_=dr_ext.ap()[ot * 128:(ot + 1) * 128, :])
                dr_sb.append(t)

            import contextlib
            _loop = tc.For_i(0, reps, 1) if reps > 1 else contextlib.nullcontext()
            with _loop:
                _body(nc, tc, const, planes, psum, work, gpool, mmp, sigpool,
                      x_sb, wt_sb, dr_sb, x_ext, wt_ext, dr_ext, out_ext)

    nc.compile()
    return nc


def _emit_planes(nc, planes, psum, x_sb, wt_sb, ot, copy_eng="act"):
    """Matmul q for this half of the channels and lay it out as the even- and
    odd-column-phase padded planes kvpe/kvpo."""
    # padded q plane, even phase: kvpe[:, r, j] = kvp[r, j]
    kvpe = planes.tile([128, H + 2, W + 2], BF16, tag="kvpe")
    # zero only the border (interior is fully overwritten)
    nc.gpsimd.memset(kvpe[:, 0, :], 0.0)
    nc.gpsimd.memset(kvpe[:, 65, :], 0.0)
    nc.gpsimd.memset(kvpe[:, 1:65, 0:1], 0.0)
    nc.gpsimd.memset(kvpe[:, 1:65, 65:66], 0.0)
    # odd phase: kvpo[:, r, j] = kvp[r, j+1]
    kvpo = planes.tile([128, H + 2, W + 2], BF16, tag="kvpo")
    nc.gpsimd.memset(kvpo[:, 0, :], 0.0)
    nc.gpsimd.memset(kvpo[:, 65, :], 0.0)
    nc.gpsimd.memset(kvpo[:, 1:65, 64:66], 0.0)

    # q = wt.T @ x in PSUM chunks of 512, ck-outer so each psum chunk
    # finishes after 2 matmuls and the copies/pointwise start early.
    pss = []
    for ck in range(HW // 512):
        ps = psum.tile([128, 512], F32, tag="ps", name=f"ps{ot}_{ck}")
        pss.append(ps)
    cpc = HW // len(x_sb[0])
    for (ib, ck) in [(ib, ck) for ck in range(HW // 512) for ib in range(2)]:
        xt = x_sb[ib][(ck * 512) // cpc]
        lo = (ck * 512) % cpc
        nc.tensor.matmul(
            pss[ck],
            wt_sb[ib][:, ot * 128:(ot + 1) * 128],
            xt[:, lo:lo + 512],
            start=(ib == 0),
            stop=(ib == 1),
        )
    for ck in range(HW // 512):
        # even-phase copy into kvpe interior band (ACT is closest to PSUM;
        # DVE via the custom copy when ACT is the busier engine)
        if copy_eng == "act" or (copy_eng == "mix" and ck % 2 == 0):
            nc.scalar.activation(
                out=kvpe[:, 1 + ck * 8:1 + (ck + 1) * 8, 1:65],
                in_=pss[ck].rearrange("p (r c) -> p r c", r=8),
                func=AF.Copy,
            )
        else:
            nc.vector._custom_dve(
                CCOPY,
                out=kvpe[:, 1 + ck * 8:1 + (ck + 1) * 8, 1:65],
                in0=pss[ck].rearrange("p (r c) -> p r c", r=8),
            )
        # odd phase band: ot0 on Pool (low latency while Pool is idle at
        # startup), ot1 on the idle DMA engines (Pool is busy with m1 by then).
        if KVPO_MODE == "dma" and ot == 1:
            nc.sync.dma_start(
                out=kvpo[:, 1 + ck * 8:1 + (ck + 1) * 8, 0:65],
                in_=kvpe[:, 1 + ck * 8:1 + (ck + 1) * 8, 1:66],
            )
        elif KVPO_MODE == "act":
            nc.scalar.activation(
                out=kvpo[:, 1 + ck * 8:1 + (ck + 1) * 8, 0:64],
                in_=pss[ck].rearrange("p (r c) -> p r c", r=8),
                func=AF.Copy,
            )
        else:  # pool
            nc.gpsimd.tensor_copy(
                out=kvpo[:, 1 + ck * 8:1 + (ck + 1) * 8, 0:65],
                in_=kvpe[:, 1 + ck * 8:1 + (ck + 1) * 8, 1:66],
            )
    return kvpe, kvpo


def _body(nc, tc, const, planes, psum, work, gpool, mmp, sigpool,
          x_sb, wt_sb, dr_sb, x_ext, wt_ext, dr_ext, out_ext):
    # Software-pipelined emission: per band, stage A (G diffs + fused arg
    # build on DVE, one batched exp on ACT) is emitted one band AHEAD of
    # stage B (fused sum+reciprocal + final combine).  Engine queues are
    # in-order, so this keeps the next band's DVE work ahead of the
    # exp-dependent ops and ACT never starves.
    BANDS0, BANDS1 = BANDS
    kvpe0, kvpo0 = _emit_planes(nc, planes, psum, x_sb, wt_sb, 0)
    sched = [(0, BANDS0[0])]
    work_items = []  # (ot, h0, ch, state)
    plan = ([(0, b) for b in BANDS0] + [(1, b) for b in BANDS1])
    kvp = {0: (kvpe0, kvpo0)}
    pending = []  # bands with A emitted, B outstanding
    for idx, (ot, (h0, ch)) in enumerate(plan):
        if ot == 1 and 1 not in kvp:
            kvp[1] = _emit_planes(nc, planes, psum, x_sb, wt_sb, 1,
                                  copy_eng=OT1_COPY_ENG)
        kvpe, kvpo = kvp[ot]
        st = _emit_band_A(nc, work, gpool, dr_sb, kvpe, kvpo, ot, h0, ch)
        pending.append((ot, h0, ch, kvpe, kvpo, st))
        # emit B for the band before the previous one (keep 2 A-stages ahead)
        if len(pending) > A_AHEAD:
            _emit_band_B(nc, mmp, sigpool, out_ext, *pending.pop(0))
    for i, item in enumerate(pending):
        _emit_band_B(nc, mmp, sigpool, out_ext, *item,
                     fast_tail=LAST_FAST and i == len(pending) - 1)


def _emit_band_A(nc, work, gpool, dr_sb, kvpe, kvpo, ot, h0, CH):
    """G diffs + fused (+-G+dr)*q arg planes (DVE custom ops) and ONE
    batched exp (ACT) for a row band.  Slot order s = 3*k + di so the
    (e0, e1) halves are contiguous for the single fused sumrecip in B."""
    RB = CH + 2
    rs = slice(h0, h0 + RB)
    g1e = gpool.tile([128, RB, W], BF16, tag="g1e")
    g1o = gpool.tile([128, RB, W], BF16, tag="g1o")
    g2e = gpool.tile([128, RB, W], BF16, tag="g2e")
    nc.vector._custom_dve(CSUB, out=g1e, in0=kvpo[:, rs, 0:64],
                          in1=kvpe[:, rs, 0:64])
    nc.vector._custom_dve(CSUB, out=g1o, in0=kvpe[:, rs, 2:66],
                          in1=kvpo[:, rs, 0:64])
    nc.vector._custom_dve(CSUB, out=g2e, in0=kvpe[:, rs, 2:66],
                          in1=kvpe[:, rs, 0:64])
    gmap = {"g1e": g1e, "g1o": g1o, "g2e": g2e}

    # q[h,w] = kvp[h+1, w+1] = kvpo[h+1, w]
    qv = kvpo[:, 1 + h0:1 + h0 + CH, 0:64]
    arg = work.tile([128, 6, CH, W], BF16, tag="arg")
    for (p, pdi, pl, ro, co, pos) in PAIRS:
        k = p % 2 if pdi == 0 else (1 if p in (3, 5) else 0)
        s = 3 * k + pdi
        gview = gmap[pl][:, ro:ro + CH, co:co + W]
        nc.vector._custom_dve(
            FARG_P if pos else FARG_N,
            out=arg[:, s], in0=gview, in1=qv,
            s0=dr_sb[ot][:, p:p + 1],
        )
    e2 = work.tile([128, 6, CH, W], BF16, tag="e2")
    nc.scalar.activation(out=e2, in_=arg, func=AF.Exp)
    return e2


def _emit_band_B(nc, mmp, sigpool, out_ext, ot, h0, CH, kvpe, kvpo, e2,
                 fast_tail=False):
    """Fused sum+reciprocal and the final diagonal-weighted combine."""
    sig_all = sigpool.tile([128, 3, CH, W], BF16, tag="sig")
    nc.vector._custom_dve(
        SUMRECIP,
        out=sig_all.rearrange("p d r c -> p (d r c)"),
        in0=e2[:, 0:3].rearrange("p d r c -> p (d r c)"),
        in1=e2[:, 3:6].rearrange("p d r c -> p (d r c)"),
        s0=RECIP_C0, s1=RECIP_C1, imm2=1.0,
    )
    v0 = kvpe[:, h0:h0 + CH, 0:64]
    v2 = kvpe[:, h0 + 2:h0 + 2 + CH, 2:66]
    qv = kvpo[:, 1 + h0:1 + h0 + CH, 0:64]
    p0 = mmp.tile([128, CH, W], BF16, tag="p0")
    p2 = mmp.tile([128, CH, W], BF16, tag="p2")
    nc.vector._custom_dve(CMUL, out=p0, in0=sig_all[:, 0], in1=v0)
    nc.vector._custom_dve(CMUL, out=p2, in0=sig_all[:, 2], in1=v2)
    m1 = mmp.tile([128, CH, W], BF16, tag="m1")
    if M1_ENG == "pool" and not fast_tail:
        nc.gpsimd.tensor_mul(m1, sig_all[:, 1], qv)
    else:
        nc.vector._custom_dve(CMUL, out=m1, in0=sig_all[:, 1], in1=qv)
    acc = mmp.tile([128, CH, W], BF16, tag="acc")
    accf = mmp.tile([128, CH, W], BF16, tag="accf")
    if ACC_ENG == "pool" and not fast_tail:
        nc.gpsimd.tensor_add(acc, p0, p2)
        nc.gpsimd.tensor_add(accf, acc, m1)
    else:
        nc.vector._custom_dve(CADD, out=acc, in0=p0, in1=p2)
        nc.vector._custom_dve(CADD, out=accf, in0=acc, in1=m1)
    nc.sync.dma_start(
        out=out_ext.ap()[ot * 128:(ot + 1) * 128,
                         h0 * W:(h0 + CH) * W],
        in_=accf.rearrange("p r c -> p (r c)"),
    )


_CACHE = {}


def _get_nc():
    if "nc" not in _CACHE:
        _CACHE["nc"] = _build()
    return _CACHE["nc"]


def _prep_in_maps(x, W_, rel):
    import ml_dtypes
    bf16 = ml_dtypes.bfloat16
    wt = np.ascontiguousarray(W_.T.astype(bf16))  # [cin, cout]
    r = rel.reshape(C, 3, 3).astype(np.float32)
    pairs = [(0, 1), (0, 2), (1, 0), (1, 2), (2, 0), (2, 1)]
    dr = np.stack([r[:, di, dj] - r[:, di, di] for (di, dj) in pairs], axis=1)
    dr = np.ascontiguousarray(dr.astype(np.float32))  # [C, 6]
    in_maps = []
    for c in range(NCORES):
        in_maps.append({
            "x": np.ascontiguousarray(x[c].reshape(C, HW).astype(bf16)),
            "wt": wt,
            "dr": dr,
        })
    return in_maps


def kernel(x, W, rel):
    nc = _get_nc()
    in_maps = _prep_in_maps(x, W, rel)
    res = run_bass_kernel_spmd(nc, in_maps, core_ids=list(range(NCORES)))
    out = np.stack([
        res.results[c]["out"].astype(np.float32).reshape(C, H, 64)
        for c in range(NCORES)
    ])
    return out.astype(np.float32)
